# revision 31
# baseline (speedup 1.0000x reference)
"""EGNN (2-layer, N=100k, E=1.6M) fully on 8 Trainium2 NeuronCores.

Single SPMD launch. Node table replicated per-core as fp16 rows
[h(64) | |x|^2 | xyz(3) | pad] (256B), built ON DEVICE before each layer
from the feature-major node state (repack phase: XBAR transpose + radial
recompute) and AllGathered. Edges partitioned by dst-owner core, grouped
by (src-quarter, dst%16 DMA channel) so SWDGE scatter-add RMWs to a row
always ride one channel FIFO. Per 4096-edge tile: 2x dma_gather
(transpose) -> edge MLP (fp16 matmuls, fp32 psum) -> XBAR dma-transpose
to edge-major payload -> dma_scatter_add(fp16). Node phase: XBAR-load of
the accumulator, node MLP + gelu + layernorm with all-ones-matmul mean
replication, output head (fp16) on core-local shards.

Host side is tuned for the axon tunnel (~50MB/s, the wall-clock
bottleneck): the PJRT executable is compiled once and cached
module-globally (run_bass_kernel_spmd rebuilds a fresh jax.jit per call,
re-paying trace+XLA-compile+NEFF-ship every launch); gather/scatter
indices ship unreplicated [16,IDXW] and are fanned out to the
[128,IDXW] SWDGE layout on device; the node table t0 is not shipped at
all (repack builds it); no donated zero output buffers (outp is fully
written); outp is fp16. Host: index packing/padding + final concat only.
"""
import os
import sys

import numpy as np

for _p in ("/opt/trn_rl_repo", "/root/.axon_site/_ro/trn_rl_repo"):
    if os.path.isdir(_p) and _p not in sys.path:
        sys.path.insert(0, _p)

N = 100000
C = 64
NOUT = 32
L = 2
NCORES = 8
SHARD = 12544
NSH = SHARD * NCORES          # 100352
SUBT = NSH // 4               # 25088
TROWS = SHARD + 128           # 12672 (trash rows 12544..12559 by channel)
QT = 13                       # tiles per src-quarter
NTL = 4 * QT                  # 52 tiles
TILE = 4096
TPC = TILE // 16              # 256 tokens per channel per tile
EPC = NTL * TILE              # 212992 edge slots per core
CH = 512
NCHK = TILE // CH             # 8
NB = SHARD // 128             # 98 node blocks
IDXW = EPC // 16              # 13312
MLPCH = 512
NMLP = SHARD // MLPCH         # 24.5 -> 24 full + one 256 tail
USE_XBAR_DRAM = True
GSUB = 512                   # idx per SWDGE call (descriptor-ring bound)
SCRATCH = 16384               # dynamic DMA descriptor carveout bytes/partition

DBG = int(os.environ.get("EGNN_DBG", "99"))      # phase bisect level
DBG_TILES = int(os.environ.get("EGNN_TILES", "0"))  # cap edge tiles (0=all)
EDGE_LVL = int(os.environ.get("EGNN_EDGE", "9"))     # edge-tile sub-bisect

# f16 weight blob layout: shipped sharded 1/8th per core, AllGathered on
# device (weights are identical across cores; replicating them in the
# upload wastes tunnel bytes). Offsets shared by host pack + device unpack.
_WOFF = {}
_off = 0
for _l in range(L):
    for _nm, _shp in (("wa", (64, 64)), ("wb", (64, 64)), ("w1rs", (4, 64)),
                      ("w2", (64, 64)), ("cw1", (64, 64)), ("cw2", (64, 1)),
                      ("nw1", (128, 64)), ("nw2", (64, 64))):
        _WOFF[f"{_nm}{_l}"] = (_off, _shp)
        _off += _shp[0] * _shp[1]
_WOFF["ow"] = (_off, (64, NOUT))
_off += 64 * NOUT
WTOT = _off                    # 60032
assert WTOT % NCORES == 0
WSH_LEN = WTOT // NCORES       # 7504


def _build_bass():
    import concourse.bacc as bacc
    import concourse.bass as bass
    import concourse.mybir as mybir
    from concourse import tile

    f16 = mybir.dt.float16
    f32 = mybir.dt.float32
    i16 = mybir.dt.int16
    i8 = mybir.dt.int8
    AF = mybir.ActivationFunctionType
    Alu = mybir.AluOpType

    nc = bacc.Bacc(None, target_bir_lowering=False, debug=False,
                   dynamic_dma_scratch_size=SCRATCH)

    # ---------------- I/O ----------------
    rx0 = nc.declare_dram_parameter("rx0", [3, SHARD], f16, isOutput=False)
    h8p = nc.declare_dram_parameter("h8", [64, SHARD], i8, isOutput=False)
    hsp = nc.declare_dram_parameter("hs", [1, SHARD], f16, isOutput=False)
    dinvp = nc.declare_dram_parameter("dinv", [1, SHARD], f16, isOutput=False)
    sidx16 = nc.declare_dram_parameter("sidx16", [16, IDXW], i16, isOutput=False)
    didx16 = nc.declare_dram_parameter("didx16", [16, IDXW], i16, isOutput=False)
    wshp = nc.declare_dram_parameter("wsh", [1, WSH_LEN], f16, isOutput=False)
    wnames = {}
    for l in range(L):
        for nm in ("b1", "b2", "cb1", "nb1", "nb2"):
            wnames[f"{nm}{l}"] = nc.declare_dram_parameter(
                f"{nm}{l}", [64, 1], f32, isOutput=False)
    ln_g = nc.declare_dram_parameter("ln_g", [64, 1], f32, isOutput=False)
    ln_b = nc.declare_dram_parameter("ln_b", [64, 1], f32, isOutput=False)
    obb = nc.declare_dram_parameter("ob", [NOUT, 1], f32, isOutput=False)
    # per-(channel, core) symmetric int8 output + abs-max scales (halves
    # the device->host fetch; host dequantizes with outsc/127)
    outp = nc.declare_dram_parameter("outp", [NOUT, SHARD], i8, isOutput=True)
    outsc = nc.declare_dram_parameter("outsc", [NOUT, 1], f32, isOutput=True)

    # ---------------- internal DRAM ----------------
    Tsh = nc.dram_tensor("Tsh", [TROWS, 128], f16)
    Tfull = nc.dram_tensor("Tfull", [NSH, 128], f16)
    ACC = nc.dram_tensor("ACCb", [TROWS, 128], f16)
    sidxw = nc.dram_tensor("sidxw", [128, IDXW], i16)
    didxw = nc.dram_tensor("didxw", [128, IDXW], i16)
    WFULL = nc.dram_tensor("WFULL", [NCORES, WSH_LEN], f16)
    WSTG = nc.dram_tensor("WSTG", [1, WSH_LEN], f16)

    with tile.TileContext(nc) as tc:
        with tc.tile_pool(name="wp", bufs=1) as wp:
            # persistent state
            RHS = wp.tile([128, SHARD], f16, tag="RHS")
            NFM = wp.tile([80, SHARD], f16, tag="NFM")
            DINVB = wp.tile([68, SHARD], f16, tag="DINVB")
            Z = wp.tile([128, 1584], f16, tag="Z")
            nc.vector.memset(Z[:], 0.0)
            # gather the sharded f16 weight blob, then unpack tiles from it
            # (collectives may not read IO tensors -> stage via internal DRAM)
            nc.sync.dma_start(WSTG[:], wshp[:])
            nc.gpsimd.collective_compute(
                "AllGather", Alu.bypass,
                replica_groups=[list(range(NCORES))],
                ins=[WSTG[:].opt()],
                outs=[WFULL[:].opt()],
            )
            WFLAT = WFULL[:].rearrange("a b -> (a b)")
            W = {}
            for name, (off, (rows, cols)) in _WOFF.items():
                src = WFLAT[off:off + rows * cols].rearrange(
                    "(p f) -> p f", p=rows)
                if name.startswith("w1rs"):
                    t = wp.tile([68, 64], f16, tag=name)
                    nc.sync.dma_start(t[64:68, :], src)
                else:
                    t = wp.tile([rows, cols], f16, tag=name)
                    nc.sync.dma_start(t[:], src)
                W[name] = t
            for l in range(L):
                for nm in ("b1", "b2", "cb1", "nb1", "nb2"):
                    h = wnames[f"{nm}{l}"]
                    t = wp.tile(list(h.shape), h.dtype, tag=f"{nm}{l}")
                    nc.sync.dma_start(t[:], h[:])
                    W[f"{nm}{l}"] = t
            for nm, h in (("ln_g", ln_g), ("ln_b", ln_b), ("ob", obb)):
                t = wp.tile(list(h.shape), h.dtype, tag=nm)
                nc.sync.dma_start(t[:], h[:])
                W[nm] = t
            ONEC = wp.tile([64, 64], f32, tag="onec")
            nc.vector.memset(ONEC[:], 1.0 / 64.0)

            # init persistent state: feature-major h arrives int8 with a
            # per-node f16 scale (halves the dominant host->device upload;
            # dequant here into RHS and NFM), [r|xyz] rows into NFM,
            # per-node 1/deg broadcast.
            with tc.tile_pool(name="hq", bufs=1) as hq:
                H8 = hq.tile([64, SHARD], i8, tag="H8")
                nc.sync.dma_start(H8[:], h8p[:])
                SCB = hq.tile([64, SHARD], f16, tag="SCB")
                nc.sync.dma_start(SCB[:], hsp[:].to_broadcast([64, SHARD]))
                nc.vector.tensor_copy(NFM[0:64, :], H8[:])
                nc.vector.tensor_tensor(NFM[0:64, :], NFM[0:64, :], SCB[:],
                                        Alu.mult)
                nc.vector.tensor_copy(RHS[0:64, :], NFM[0:64, :])
            # xyz rows only; row 64 (radial) is recomputed by every repack
            nc.sync.dma_start(NFM[65:68, :], rx0[:])
            nc.sync.dma_start(DINVB[64:68, :], dinvp[:].to_broadcast([4, SHARD]))
            # zero the per-channel trash rows of the dst-gather table once
            nc.sync.dma_start(
                Tsh[SHARD:TROWS, :], Z[:, 0:128])
            # fan the [16, IDXW] index uploads out to the SWDGE 128-row form
            for k in range(8):
                nc.sync.dma_start(sidxw[16 * k:16 * (k + 1), :], sidx16[:])
                nc.sync.dma_start(didxw[16 * k:16 * (k + 1), :], didx16[:])

            for l in range(L):
                if DBG >= 1 + 3 * l:
                    _repack_phase(nc, tc, bass, mybir, NFM, Tsh, Tfull, ACC, Z)
                if DBG >= 2 + 3 * l:
                    _edge_phase(nc, tc, bass, mybir, l, W, Tsh, Tfull, ACC,
                                sidxw, didxw)
                if DBG >= 3 + 3 * l:
                    _node_phase(nc, tc, bass, mybir, l, W, ACC, RHS, NFM, DINVB,
                                ONEC, outp, outsc)
                if DBG < 4 and l == 0:
                    break
            if DBG < 6:
                # ensure outputs are written so they exist
                with tc.tile_pool(name="dbgo", bufs=1) as dp:
                    zo = dp.tile([NOUT, 512], i8, tag="zo")
                    nc.vector.memset(zo[:], 0.0)
                    for k in range(SHARD // 512 + 1):
                        st = min(k * 512, SHARD - 512)
                        nc.sync.dma_start(outp[:, st:st + 512], zo[:])
                    zs = dp.tile([NOUT, 1], f32, tag="zs")
                    nc.vector.memset(zs[:], 127.0)
                    nc.sync.dma_start(outsc[:], zs[:])

    nc.finalize()
    return nc


def _edge_phase(nc, tc, bass, mybir, l, W, Tsh, Tfull, ACC, sidxw, didxw):
    from concourse import tile  # noqa: F401
    f16 = mybir.dt.float16
    f32 = mybir.dt.float32
    i16 = mybir.dt.int16
    AF = mybir.ActivationFunctionType
    Alu = mybir.AluOpType
    wa, wb, w1rs = W[f"wa{l}"], W[f"wb{l}"], W[f"w1rs{l}"]
    w2, cw1, cw2 = W[f"w2{l}"], W[f"cw1{l}"], W[f"cw2{l}"]
    b1, b2, cb1 = W[f"b1{l}"], W[f"b2{l}"], W[f"cb1{l}"]

    with (
        tc.tile_pool(name=f"eio{l}", bufs=2) as io,
        tc.tile_pool(name=f"emp{l}", bufs=2) as mp,
        tc.tile_pool(name=f"eps{l}", bufs=2, space=bass.MemorySpace.PSUM) as pp,
        tc.tile_pool(name=f"ep4{l}", bufs=2, space=bass.MemorySpace.PSUM) as p4p,
    ):
        ntl = min(NTL, DBG_TILES) if DBG_TILES else NTL
        for t in range(ntl):
            q = t // QT
            csl = slice(t * TPC, (t + 1) * TPC)
            sid = io.tile([128, TPC], i16, tag="sid")
            did = io.tile([128, TPC], i16, tag="did")
            nc.sync.dma_start(sid[:], sidxw[:, csl])
            nc.sync.dma_start(did[:], didxw[:, csl])

            Gs = io.tile([128, TILE], f16, tag="Gs")
            Gd = io.tile([128, TILE], f16, tag="Gd")
            for s in range(TILE // GSUB):
                isl = slice(s * (GSUB // 16), (s + 1) * (GSUB // 16))
                esl = slice(s * GSUB, (s + 1) * GSUB)
                nc.gpsimd.dma_gather(
                    Gs[:, esl].rearrange("p (a f) -> p a f", a=1),
                    Tfull[q * SUBT:(q + 1) * SUBT, :],
                    sid[:, isl], GSUB, GSUB, 128, transpose=True)
                nc.gpsimd.dma_gather(
                    Gd[:, esl].rearrange("p (a f) -> p a f", a=1),
                    Tsh[:], did[:, isl], GSUB, GSUB, 128, transpose=True)

            if EDGE_LVL < 2:
                continue
            # radial terms (feature-major) for MM1: row64 = rs+rd, 65:68 = xs*xd
            pxd = mp.tile([68, TILE], f16, tag="pxd")
            nc.vector.tensor_tensor(pxd[64:68, :], Gs[64:68, :], Gd[64:68, :], Alu.mult)
            nc.vector.tensor_tensor(pxd[64:65, :], Gs[64:65, :], Gd[64:65, :], Alu.add)

            # edge-major xyz via XBAR transpose
            XS = mp.tile([128, TILE // 128, 16], f16, tag="XS")
            XD = mp.tile([128, TILE // 128, 16], f16, tag="XD")
            nc.sync.dma_start(XS[:], Gs[64:80, :], transpose=True)
            nc.sync.dma_start(XD[:], Gd[64:80, :], transpose=True)
            dxT = mp.tile([128, TILE // 128, 3], f32, tag="dxT")
            nc.vector.tensor_tensor(dxT[:], XS[:, :, 1:4], XD[:, :, 1:4], Alu.subtract)
            sq = mp.tile([128, TILE // 128, 3], f32, tag="sq")
            nc.vector.tensor_tensor(sq[:], dxT[:], dxT[:], Alu.mult)
            rad = mp.tile([128, TILE // 128], f32, tag="rad")
            nc.vector.tensor_tensor(rad[:], sq[:, :, 0], sq[:, :, 1], Alu.add)
            nc.vector.tensor_tensor(rad[:], rad[:], sq[:, :, 2], Alu.add)
            sr = mp.tile([128, TILE // 128], f32, tag="sr")
            nc.scalar.activation(sr[:], rad[:], AF.Sqrt)
            nc.vector.tensor_scalar(sr[:], sr[:], 1e-30, None, Alu.add)
            inv = mp.tile([128, TILE // 128], f32, tag="inv")
            nc.vector.reciprocal(inv[:], sr[:])

            if EDGE_LVL < 3:
                continue
            MSG = mp.tile([64, TILE], f16, tag="MSG")
            CS16 = mp.tile([16, TILE], f16, tag="CS16")
            for k in range(NCHK):
                sl = slice(k * CH, (k + 1) * CH)
                P1 = pp.tile([64, CH], f32, tag="pb")
                nc.tensor.matmul(P1[:], wa[:], Gs[0:64, sl], start=True, stop=False)
                nc.tensor.matmul(P1[:], wb[:], Gd[0:64, sl], start=False, stop=False)
                nc.tensor.matmul(P1[:], w1rs[64:68, :], pxd[64:68, sl],
                                 start=False, stop=True, tile_position=(64, 0))
                S1 = mp.tile([64, CH], f16, tag="s1")
                nc.scalar.activation(S1[:], P1[:], AF.Silu, bias=b1[:])
                P2 = pp.tile([64, CH], f32, tag="pb")
                nc.tensor.matmul(P2[:], w2[:], S1[:], start=True, stop=True)
                nc.scalar.activation(MSG[:, sl], P2[:], AF.Silu, bias=b2[:])
                P3 = pp.tile([64, CH], f32, tag="pb")
                nc.tensor.matmul(P3[:], cw1[:], MSG[:, sl], start=True, stop=True)
                S3 = mp.tile([64, CH], f16, tag="s3")
                nc.scalar.activation(S3[:], P3[:], AF.Silu, bias=cb1[:])
                P4 = p4p.tile([1, CH], f32, tag="p4")
                nc.tensor.matmul(P4[:], cw2[:], S3[:], start=True, stop=True)
                nc.vector.tensor_copy(CS16[0:1, sl], P4[:])

            if EDGE_LVL < 4:
                continue
            CST = mp.tile([128, TILE // 128, 16], f16, tag="CST")
            nc.sync.dma_start(CST[:], CS16[:], transpose=True)
            cs32 = mp.tile([128, TILE // 128], f32, tag="cs32")
            nc.vector.tensor_copy(cs32[:], CST[:, :, 0])
            nc.vector.tensor_tensor(cs32[:], cs32[:], inv[:], Alu.mult)

            if EDGE_LVL < 5:
                continue
            PAY = mp.tile([128, TILE // 128, 128], f16, tag="PAY")
            nc.vector.tensor_tensor(
                PAY[:, :, 65:68], dxT[:],
                cs32[:].rearrange("p (a b) -> p a b", b=1).to_broadcast(
                    [128, TILE // 128, 3]),
                Alu.mult)
            nc.sync.dma_start(PAY[:, :, 0:64], MSG[:], transpose=True)
            for s in range(TILE // GSUB):
                nc.gpsimd.dma_scatter_add(
                    ACC[:], PAY[:, s * (GSUB // 128):(s + 1) * (GSUB // 128), :],
                    did[:, s * (GSUB // 16):(s + 1) * (GSUB // 16)],
                    GSUB, GSUB, 128)


def _node_phase(nc, tc, bass, mybir, l, W, ACC, RHS, NFM, DINVB, ONEC, outp,
                outsc):
    f16 = mybir.dt.float16
    f32 = mybir.dt.float32
    i8 = mybir.dt.int8
    AF = mybir.ActivationFunctionType
    Alu = mybir.AluOpType
    nw1, nw2 = W[f"nw1{l}"], W[f"nw2{l}"]
    nb1, nb2 = W[f"nb1{l}"], W[f"nb2{l}"]

    with (
        tc.tile_pool(name=f"nd{l}", bufs=3) as nd,
        tc.tile_pool(name=f"no{l}", bufs=1) as op_,
        tc.tile_pool(name=f"nps{l}", bufs=2, space=bass.MemorySpace.PSUM) as pp,
        tc.tile_pool(name=f"np2{l}", bufs=2, space=bass.MemorySpace.PSUM) as pp2,
    ):
        OALL = None
        if l == L - 1:
            OALL = op_.tile([NOUT, SHARD], f16, tag="OALL")
        for b in range(NB):
            bsl = slice(b * 128, (b + 1) * 128)
            if USE_XBAR_DRAM:
                ABT = nd.tile([128, 128], f16, tag="ABT")
                nc.sync.dma_start(ABT[:], ACC[bsl, :], transpose=True)
            else:
                AB = nd.tile([128, 128], f16, tag="AB")
                nc.sync.dma_start(AB[:], ACC[bsl, :])
                ABT = nd.tile([128, 128], f16, tag="ABT")
                nc.sync.dma_start(ABT[:], AB[:], transpose=True)
            nc.sync.dma_start(RHS[64:128, bsl], ABT[0:64, :])
            if l == 0:
                xn = nd.tile([68, 128], f16, tag="xn")
                nc.vector.tensor_tensor(
                    xn[64:68, :], ABT[64:68, :], DINVB[64:68, bsl], Alu.mult)
                nc.vector.tensor_tensor(
                    NFM[64:68, bsl], NFM[64:68, bsl], xn[64:68, :], Alu.add)

        # node MLP + gelu + LN (+ output head on last layer)
        nchunks = [(k * MLPCH, MLPCH) for k in range(NMLP)]
        if NMLP * MLPCH < SHARD:
            nchunks.append((NMLP * MLPCH, SHARD - NMLP * MLPCH))
        for (st, ln_) in nchunks:
            sl = slice(st, st + ln_)
            P = pp.tile([64, ln_], f32, tag="pn")
            nc.tensor.matmul(P[:], nw1[:], RHS[:, sl], start=True, stop=True)
            S = nd.tile([64, ln_], f16, tag="sn")
            nc.scalar.activation(S[:], P[:], AF.Silu, bias=nb1[:])
            P2 = pp.tile([64, ln_], f32, tag="pn")
            nc.tensor.matmul(P2[:], nw2[:], S[:], start=True, stop=True)
            H2 = nd.tile([64, ln_], f32, tag="h2")
            nc.scalar.activation(H2[:], P2[:], AF.Gelu, bias=nb2[:])
            SQH = nd.tile([64, ln_], f32, tag="sqh")
            nc.scalar.activation(SQH[:], H2[:], AF.Square)
            MUR = pp2.tile([64, ln_], f32, tag="mur")
            nc.tensor.matmul(MUR[:], ONEC[:], H2[:], start=True, stop=True)
            MU = nd.tile([64, ln_], f32, tag="mu")
            nc.vector.tensor_copy(MU[:], MUR[:])
            E2R = pp2.tile([64, ln_], f32, tag="mur")
            nc.tensor.matmul(E2R[:], ONEC[:], SQH[:], start=True, stop=True)
            VAR = nd.tile([64, ln_], f32, tag="var")
            nc.vector.tensor_tensor(VAR[:], MU[:], MU[:], Alu.mult)
            nc.vector.tensor_tensor(VAR[:], E2R[:], VAR[:], Alu.subtract)
            nc.vector.tensor_scalar(VAR[:], VAR[:], 1e-5, None, Alu.add)
            SD = nd.tile([64, ln_], f32, tag="sd")
            nc.scalar.activation(SD[:], VAR[:], AF.Sqrt)
            ISD = nd.tile([64, ln_], f32, tag="isd")
            nc.vector.reciprocal(ISD[:], SD[:])
            HC = nd.tile([64, ln_], f32, tag="hc")
            nc.vector.tensor_tensor(HC[:], H2[:], MU[:], Alu.subtract)
            nc.vector.tensor_tensor(HC[:], HC[:], ISD[:], Alu.mult)
            HFIN = nd.tile([64, ln_], f32, tag="hfin")
            nc.vector.tensor_scalar(
                HFIN[:], HC[:], W["ln_g"][:], W["ln_b"][:], Alu.mult, Alu.add)
            if l == 0:
                nc.vector.tensor_copy(NFM[0:64, sl], HFIN[:])
                nc.vector.tensor_copy(RHS[0:64, sl], HFIN[:])
            else:
                HF16 = nd.tile([64, ln_], f16, tag="hf16")
                nc.vector.tensor_copy(HF16[:], HFIN[:])
                PO = pp.tile([NOUT, ln_], f32, tag="po")
                nc.tensor.matmul(PO[:], W["ow"][:], HF16[:], start=True, stop=True)
                nc.scalar.activation(OALL[:, sl], PO[:], AF.Identity,
                                     bias=W["ob"][:])

        if l == L - 1:
            # per-channel abs-max -> symmetric int8 (round-to-nearest,
            # saturating); host dequantizes with outsc/127
            MX = op_.tile([NOUT, 1], f32, tag="MX")
            nc.vector.tensor_reduce(MX[:], OALL[:], mybir.AxisListType.X,
                                    Alu.max, apply_absolute_value=True)
            nc.vector.tensor_scalar(MX[:], MX[:], 1e-20, None, Alu.max)
            SC = op_.tile([NOUT, 1], f32, tag="SC")
            nc.vector.reciprocal(SC[:], MX[:])
            nc.vector.tensor_scalar(SC[:], SC[:], 127.0, None, Alu.mult)
            OI8 = op_.tile([NOUT, SHARD], i8, tag="OI8")
            nc.vector.tensor_scalar(OI8[:], OALL[:], SC[:], None, Alu.mult)
            nc.sync.dma_start(outp[:], OI8[:])
            nc.sync.dma_start(outsc[:], MX[:])


def _repack_phase(nc, tc, bass, mybir, NFM, Tsh, Tfull, ACC, Z):
    f16 = mybir.dt.float16
    f32 = mybir.dt.float32
    Alu = mybir.AluOpType
    with tc.tile_pool(name="pk", bufs=1) as pk:
        PK = pk.tile([128, NB, 128], f16, tag="PK")
        nc.vector.memset(PK[:], 0.0)
        nc.sync.dma_start(PK[:, :, 0:80], NFM[:, :], transpose=True)
        TSQ = pk.tile([128, NB, 3], f32, tag="TSQ")
        nc.vector.tensor_tensor(TSQ[:], PK[:, :, 65:68], PK[:, :, 65:68], Alu.mult)
        R2 = pk.tile([128, NB], f32, tag="R2")
        nc.vector.tensor_tensor(R2[:], TSQ[:, :, 0], TSQ[:, :, 1], Alu.add)
        nc.vector.tensor_tensor(R2[:], R2[:], TSQ[:, :, 2], Alu.add)
        nc.vector.tensor_copy(PK[:, :, 64], R2[:])
        nc.sync.dma_start(
            Tsh[0:SHARD, :].rearrange("(b p) e -> p b e", p=128), PK[:])
        for i in range(8):
            nc.sync.dma_start(
                ACC[:].rearrange("(a r) e -> a (r e)", a=8)[i:i + 1, :]
                .rearrange("a (p f) -> (a p) f", p=128),
                Z[:])
        nc.gpsimd.collective_compute(
            "AllGather", Alu.bypass,
            replica_groups=[list(range(NCORES))],
            ins=[Tsh[0:SHARD, :].opt()],
            outs=[Tfull[:].opt()],
        )


# ---------------- host runner (cached PJRT executable) ----------------
_RUNNER = None
_NC = None


def _get_nc():
    global _NC
    if _NC is None:
        _NC = _build_bass()
    return _NC


def _get_runner():
    """Build the Bass module once, lower it through the bass_exec custom
    call, and cache the jitted SPMD executable. run_bass_kernel_spmd's
    axon path (bass2jax.run_bass_via_pjrt) constructs a fresh jax.jit on
    every call, re-paying trace + XLA compile + NEFF ship per launch;
    this is the same lowering with the jit hoisted. No donated zero
    output buffers: outp is fully written by the kernel."""
    global _RUNNER
    if _RUNNER is not None:
        return _RUNNER

    import jax
    from jax.experimental.shard_map import shard_map
    from jax.sharding import Mesh, PartitionSpec

    import concourse.bass2jax as b2j
    import concourse.mybir as mybir

    nc = _get_nc()
    b2j.install_neuronx_cc_hook()

    partition_name = nc.partition_id_tensor.name if nc.partition_id_tensor else None
    in_names, out_names, out_avals = [], [], []
    for alloc in nc.m.functions[0].allocations:
        if not isinstance(alloc, mybir.MemoryLocationSet):
            continue
        name = alloc.memorylocations[0].name
        if alloc.kind == "ExternalInput":
            if name != partition_name:
                in_names.append(name)
        elif alloc.kind == "ExternalOutput":
            out_avals.append(jax.core.ShapedArray(
                tuple(alloc.tensor_shape), mybir.dt.np(alloc.dtype)))
            out_names.append(name)
    bind_names = tuple(in_names) + ((partition_name,) if partition_name else ())

    def _body(*args):
        operands = list(args)
        if partition_name is not None:
            operands.append(b2j.partition_id_tensor())
        return tuple(b2j._bass_exec_p.bind(
            *operands,
            out_avals=tuple(out_avals),
            in_names=bind_names,
            out_names=tuple(out_names),
            lowering_input_output_aliases=(),
            sim_require_finite=True,
            sim_require_nnan=True,
            nc=nc,
        ))

    devices = jax.devices()[:NCORES]
    assert len(devices) == NCORES, f"need {NCORES} cores, have {len(jax.devices())}"
    mesh = Mesh(np.asarray(devices), ("core",))
    sharded = jax.jit(
        shard_map(_body, mesh=mesh,
                  in_specs=(PartitionSpec("core"),) * len(in_names),
                  out_specs=(PartitionSpec("core"),) * len(out_names),
                  check_rep=False),
        keep_unused=True)
    _RUNNER = (sharded, in_names, out_names)
    return _RUNNER


def _run_device(in_maps):
    if isinstance(in_maps, dict):
        per_core, concat = in_maps["maps"], in_maps["concat"]
    else:
        per_core, concat = in_maps, None
    try:
        sharded, in_names, out_names = _get_runner()
        if concat is None:
            concat = {
                name: np.concatenate(
                    [np.asarray(per_core[c][name]) for c in range(NCORES)],
                    axis=0)
                for name in in_names
            }
        out_arrs = sharded(*[concat[name] for name in in_names])
        qi8 = np.asarray(out_arrs[out_names.index("outp")])   # [8*32, SHARD] i8
        scl = np.asarray(out_arrs[out_names.index("outsc")])  # [8*32, 1] f32
    except Exception:
        # cached-jit path failed (internal bass2jax API drift?) — fall back
        # to the stock per-call runner; slower but identical device program
        from concourse.bass_utils import run_bass_kernel_spmd
        res = run_bass_kernel_spmd(_get_nc(), per_core,
                                   core_ids=list(range(NCORES)))
        qi8 = np.concatenate(
            [np.asarray(res.results[c]["outp"]) for c in range(NCORES)], axis=0)
        scl = np.concatenate(
            [np.asarray(res.results[c]["outsc"]) for c in range(NCORES)], axis=0)
    deq = qi8.astype(np.float32)
    deq *= scl.astype(np.float32) / 127.0
    out = deq.reshape(NCORES, NOUT, SHARD).transpose(0, 2, 1).reshape(NSH, NOUT)
    return out[:N]


def _pack_inputs(node_feat, xyz, src, dst, weights):
    """Host-side preprocessing -> per-core in_maps."""
    (edge_w1, edge_b1, edge_w2, edge_b2, coord_w1, coord_b1, coord_w2,
     node_w1, node_b1, node_w2, node_b2, ln_g, ln_b, out_w, out_b) = weights

    # per-node symmetric int8 quantization of h (dequantized on device)
    hsc = np.maximum(np.abs(node_feat).max(1, keepdims=True) / 127.0, 1e-8)
    hq8 = np.clip(np.round(node_feat / hsc), -127, 127).astype(np.int8)
    x16 = xyz.astype(np.float16)
    deg = np.bincount(dst, minlength=N).astype(np.float32)
    dinv = (1.0 / np.maximum(deg, 1.0)).astype(np.float16)

    # pack the f16 weight blob once; each core ships 1/8th of it
    wblob = np.zeros(WTOT, np.float16)
    wsrc = {}
    for l in range(L):
        w1 = edge_w1[l].astype(np.float32)
        wr = w1[128]
        wsrc[f"wa{l}"] = w1[0:64]
        wsrc[f"wb{l}"] = w1[64:128]
        wsrc[f"w1rs{l}"] = np.concatenate(
            [wr[None], np.tile((-2.0 * wr)[None], (3, 1))], 0)
        wsrc[f"w2{l}"] = edge_w2[l]
        wsrc[f"cw1{l}"] = coord_w1[l]
        wsrc[f"cw2{l}"] = coord_w2[l]
        wsrc[f"nw1{l}"] = node_w1[l]
        wsrc[f"nw2{l}"] = node_w2[l]
    wsrc["ow"] = out_w
    for name, (off, shp) in _WOFF.items():
        wblob[off:off + shp[0] * shp[1]] = (
            wsrc[name].astype(np.float16).reshape(-1))

    core = dst // SHARD
    qq = src // SUBT
    chan = dst % 16
    order = np.lexsort((dst, chan, qq, core))
    src_s, dst_s = src[order], dst[order]

    # group boundaries per (core, q, chan)
    key = (core[order] * 4 + qq[order]) * 16 + chan[order]
    cnt = np.bincount(key, minlength=NCORES * 4 * 16)
    assert cnt.max() <= QT * TPC, f"group overflow: {cnt.max()} > {QT * TPC}"
    starts = np.concatenate(([0], np.cumsum(cnt)))

    in_maps = []
    for c in range(NCORES):
        lo = c * SHARD
        hi = min(lo + SHARD, N)
        nreal = hi - lo

        rx0 = np.zeros((3, SHARD), np.float16)
        rx0[:, :nreal] = x16[lo:hi].T

        h8 = np.zeros((64, SHARD), np.int8)
        h8[:, :nreal] = hq8[lo:hi].T
        hs = np.zeros((1, SHARD), np.float16)
        hs[0, :nreal] = hsc[lo:hi, 0].astype(np.float16)

        dv = np.ones((1, SHARD), np.float16)
        dv[0, :nreal] = dinv[lo:hi]

        # idx packing: A[tile, chan(16), pos(TPC)]
        As = np.zeros((NTL, 16, TPC), np.int16)
        Ad = np.full((NTL, 16, TPC), 0, np.int16)
        for q in range(4):
            for ch in range(16):
                g = (c * 4 + q) * 16 + ch
                e0, e1 = starts[g], starts[g] + cnt[g]
                sl_src = (src_s[e0:e1] - q * SUBT).astype(np.int16)
                sl_dst = (dst_s[e0:e1] - lo).astype(np.int16)
                nfull = cnt[g]
                base_t = q * QT
                # Round-robin each dst's edges over all QT*8 (tile,
                # scatter-subcall) slots. Edges of one dst are consecutive
                # in the sorted order, so occurrences land in distinct
                # tiles (k<=13) and beyond that in distinct subcalls >=32
                # descriptors apart in the channel FIFO — duplicate idx
                # within one dma_scatter_add call race the f16 RMW
                # pipeline (measured: ~150 corrupted nodes/run without
                # this).
                NSLOT = QT * NCHK          # 104
                BCAP = TPC // NCHK         # 32 slots per (tile, subcall)
                As[base_t:base_t + QT, ch, :] = 0
                Ad[base_t:base_t + QT, ch, :] = SHARD + ch
                if nfull > 0:
                    s_ = np.arange(nfull) % NSLOT
                    t_of, u_of = s_ % QT, s_ // QT
                    # rank within each (tile, subcall) bucket
                    bucket = t_of * NCHK + u_of
                    order2 = np.argsort(bucket, kind="stable")
                    bsort = bucket[order2]
                    bstart = np.searchsorted(bsort, np.arange(NSLOT))
                    rank = np.arange(nfull) - bstart[bsort]
                    assert rank.max() < BCAP, rank.max()
                    col = u_of[order2] * BCAP + rank
                    As[base_t + t_of[order2], ch, col] = sl_src[order2]
                    Ad[base_t + t_of[order2], ch, col] = sl_dst[order2]
        # [16, IDXW]: rows = 16-wrap; tile t occupies cols [t*TPC, (t+1)*TPC)
        Ws_ = As.transpose(1, 0, 2).reshape(16, NTL * TPC)
        Wd_ = Ad.transpose(1, 0, 2).reshape(16, NTL * TPC)

        m = {
            "rx0": rx0, "h8": h8, "hs": hs, "dinv": dv,
            "sidx16": np.ascontiguousarray(Ws_),
            "didx16": np.ascontiguousarray(Wd_),
            "wsh": wblob[c * WSH_LEN:(c + 1) * WSH_LEN].reshape(1, WSH_LEN),
            "ln_g": ln_g.reshape(64, 1).astype(np.float32),
            "ln_b": ln_b.reshape(64, 1).astype(np.float32),
            "ob": out_b.reshape(NOUT, 1).astype(np.float32),
        }
        for l in range(L):
            m[f"b1{l}"] = edge_b1[l].reshape(64, 1).astype(np.float32)
            m[f"b2{l}"] = edge_b2[l].reshape(64, 1).astype(np.float32)
            m[f"cb1{l}"] = coord_b1[l].reshape(64, 1).astype(np.float32)
            m[f"nb1{l}"] = node_b1[l].reshape(64, 1).astype(np.float32)
            m[f"nb2{l}"] = node_b2[l].reshape(64, 1).astype(np.float32)
        in_maps.append(m)
    # pre-concatenate into the shard_map global layout (untimed host prep)
    concat = {
        name: np.ascontiguousarray(
            np.concatenate([in_maps[c][name] for c in range(NCORES)], axis=0))
        for name in in_maps[0]
    }
    return {"maps": in_maps, "concat": concat}


def kernel(node_feat, xyz, src, dst, edge_w1, edge_b1, edge_w2, edge_b2,
           coord_w1, coord_b1, coord_w2, node_w1, node_b1, node_w2, node_b2,
           ln_g, ln_b, out_w, out_b):
    node_feat = np.asarray(node_feat, np.float32)
    xyz = np.asarray(xyz, np.float32)
    src = np.asarray(src, np.int32)
    dst = np.asarray(dst, np.int32)
    weights = (np.asarray(edge_w1, np.float32), np.asarray(edge_b1, np.float32),
               np.asarray(edge_w2, np.float32), np.asarray(edge_b2, np.float32),
               np.asarray(coord_w1, np.float32), np.asarray(coord_b1, np.float32),
               np.asarray(coord_w2, np.float32),
               np.asarray(node_w1, np.float32), np.asarray(node_b1, np.float32),
               np.asarray(node_w2, np.float32), np.asarray(node_b2, np.float32),
               np.asarray(ln_g, np.float32), np.asarray(ln_b, np.float32),
               np.asarray(out_w, np.float32), np.asarray(out_b, np.float32))
    in_maps = _pack_inputs(node_feat, xyz, src, dst, weights)
    return _run_device(in_maps)


# revision 37
# speedup vs baseline: 1.2335x; 1.2335x over previous
"""EGNN (2-layer, N=100k, E=1.6M) fully on 8 Trainium2 NeuronCores.

Single SPMD launch. Node table replicated per-core as fp16 rows
[h(64) | |x|^2 | xyz(3) | pad] (256B), built ON DEVICE before each layer
from the feature-major node state (repack phase: XBAR transpose + radial
recompute) and AllGathered. Edges partitioned by dst-owner core, grouped
by (src-quarter, dst%16 DMA channel) so SWDGE scatter-add RMWs to a row
always ride one channel FIFO. Per 4096-edge tile: 2x dma_gather
(transpose) -> edge MLP (fp16 matmuls, fp32 psum) -> XBAR dma-transpose
to edge-major payload -> dma_scatter_add(fp16). Node phase: XBAR-load of
the accumulator, node MLP + gelu + layernorm with all-ones-matmul mean
replication, output head (fp16) on core-local shards.

Host side is tuned for the axon tunnel (~50MB/s, the wall-clock
bottleneck): the PJRT executable is compiled once and cached
module-globally (run_bass_kernel_spmd rebuilds a fresh jax.jit per call,
re-paying trace+XLA-compile+NEFF-ship every launch); gather/scatter
indices ship unreplicated [16,IDXW] and are fanned out to the
[128,IDXW] SWDGE layout on device; the node table t0 is not shipped at
all (repack builds it); no donated zero output buffers (outp is fully
written); outp is fp16. Host: index packing/padding + final concat only.
"""
import os
import sys

import numpy as np

for _p in ("/opt/trn_rl_repo", "/root/.axon_site/_ro/trn_rl_repo"):
    if os.path.isdir(_p) and _p not in sys.path:
        sys.path.insert(0, _p)

N = 100000
C = 64
NOUT = 32
L = 2
NCORES = 8
SHARD = 12544
NSH = SHARD * NCORES          # 100352
SUBT = NSH // 4               # 25088
TROWS = SHARD + 128           # 12672 (trash rows 12544..12559 by channel)
QT = 13                       # tiles per src-quarter
NTL = 4 * QT                  # 52 tiles
TILE = 4096
TPC = TILE // 16              # 256 tokens per channel per tile
EPC = NTL * TILE              # 212992 edge slots per core
CH = 512
NCHK = TILE // CH             # 8
NB = SHARD // 128             # 98 node blocks
IDXW = EPC // 16              # 13312
MLPCH = 512
NMLP = SHARD // MLPCH         # 24.5 -> 24 full + one 256 tail
USE_XBAR_DRAM = True
GSUB = 512                   # idx per SWDGE call (descriptor-ring bound)
SCRATCH = 16384               # dynamic DMA descriptor carveout bytes/partition

DBG = int(os.environ.get("EGNN_DBG", "99"))      # phase bisect level
DBG_TILES = int(os.environ.get("EGNN_TILES", "0"))  # cap edge tiles (0=all)
EDGE_LVL = int(os.environ.get("EGNN_EDGE", "9"))     # edge-tile sub-bisect

# f16 weight blob layout: shipped sharded 1/8th per core, AllGathered on
# device (weights are identical across cores; replicating them in the
# upload wastes tunnel bytes). Offsets shared by host pack + device unpack.
_WOFF = {}
_off = 0
for _l in range(L):
    for _nm, _shp in (("wa", (64, 64)), ("wb", (64, 64)), ("w1rs", (4, 64)),
                      ("w2", (64, 64)), ("cw1", (64, 64)), ("cw2", (64, 1)),
                      ("nw1", (128, 64)), ("nw2", (64, 64))):
        _WOFF[f"{_nm}{_l}"] = (_off, _shp)
        _off += _shp[0] * _shp[1]
_WOFF["ow"] = (_off, (64, NOUT))
_off += 64 * NOUT
WTOT = _off                    # 60032
assert WTOT % NCORES == 0
WSH_LEN = WTOT // NCORES       # 7504


def _build_bass():
    import concourse.bacc as bacc
    import concourse.bass as bass
    import concourse.mybir as mybir
    from concourse import tile

    f16 = mybir.dt.float16
    f32 = mybir.dt.float32
    i16 = mybir.dt.int16
    i8 = mybir.dt.int8
    AF = mybir.ActivationFunctionType
    Alu = mybir.AluOpType

    nc = bacc.Bacc(None, target_bir_lowering=False, debug=False,
                   dynamic_dma_scratch_size=SCRATCH)

    # ---------------- I/O ----------------
    rx0 = nc.declare_dram_parameter("rx0", [3, SHARD], f16, isOutput=False)
    h8p = nc.declare_dram_parameter("h8", [64, SHARD], i8, isOutput=False)
    hsp = nc.declare_dram_parameter("hs", [1, SHARD], f16, isOutput=False)
    dinvp = nc.declare_dram_parameter("dinv", [1, SHARD], f16, isOutput=False)
    sidx16 = nc.declare_dram_parameter("sidx16", [16, IDXW], i16, isOutput=False)
    didx16 = nc.declare_dram_parameter("didx16", [16, IDXW], i16, isOutput=False)
    wshp = nc.declare_dram_parameter("wsh", [1, WSH_LEN], f16, isOutput=False)
    wnames = {}
    for l in range(L):
        for nm in ("b1", "b2", "cb1", "nb1", "nb2"):
            wnames[f"{nm}{l}"] = nc.declare_dram_parameter(
                f"{nm}{l}", [64, 1], f32, isOutput=False)
    ln_g = nc.declare_dram_parameter("ln_g", [64, 1], f32, isOutput=False)
    ln_b = nc.declare_dram_parameter("ln_b", [64, 1], f32, isOutput=False)
    obb = nc.declare_dram_parameter("ob", [NOUT, 1], f32, isOutput=False)
    # per-(channel, core) symmetric int8 output (halves the device->host
    # fetch); the f32 abs-max scale rides in the last 4 columns of each row
    # (bitcast) so the host pays a single fetch. Host dequantizes /127.
    outp = nc.declare_dram_parameter("outp", [NOUT, SHARD + 4], i8, isOutput=True)

    # ---------------- internal DRAM ----------------
    Tsh = nc.dram_tensor("Tsh", [TROWS, 128], f16)
    Tfull = nc.dram_tensor("Tfull", [NSH, 128], f16)
    ACC = nc.dram_tensor("ACCb", [TROWS, 128], f16)
    sidxw = nc.dram_tensor("sidxw", [128, IDXW], i16)
    didxw = nc.dram_tensor("didxw", [128, IDXW], i16)
    WFULL = nc.dram_tensor("WFULL", [NCORES, WSH_LEN], f16)
    WSTG = nc.dram_tensor("WSTG", [1, WSH_LEN], f16)

    with tile.TileContext(nc) as tc:
        with tc.tile_pool(name="wp", bufs=1) as wp:
            # persistent state
            RHS = wp.tile([128, SHARD], f16, tag="RHS")
            NFM = wp.tile([80, SHARD], f16, tag="NFM")
            DINVB = wp.tile([68, SHARD], f16, tag="DINVB")
            Z = wp.tile([128, 1584], f16, tag="Z")
            nc.vector.memset(Z[:], 0.0)
            # gather the sharded f16 weight blob, then unpack tiles from it
            # (collectives may not read IO tensors -> stage via internal DRAM)
            nc.sync.dma_start(WSTG[:], wshp[:])
            nc.gpsimd.collective_compute(
                "AllGather", Alu.bypass,
                replica_groups=[list(range(NCORES))],
                ins=[WSTG[:].opt()],
                outs=[WFULL[:].opt()],
            )
            WFLAT = WFULL[:].rearrange("a b -> (a b)")
            W = {}
            for name, (off, (rows, cols)) in _WOFF.items():
                src = WFLAT[off:off + rows * cols].rearrange(
                    "(p f) -> p f", p=rows)
                if name.startswith("w1rs"):
                    t = wp.tile([68, 64], f16, tag=name)
                    nc.sync.dma_start(t[64:68, :], src)
                else:
                    t = wp.tile([rows, cols], f16, tag=name)
                    nc.sync.dma_start(t[:], src)
                W[name] = t
            for l in range(L):
                for nm in ("b1", "b2", "cb1", "nb1", "nb2"):
                    h = wnames[f"{nm}{l}"]
                    t = wp.tile(list(h.shape), h.dtype, tag=f"{nm}{l}")
                    nc.sync.dma_start(t[:], h[:])
                    W[f"{nm}{l}"] = t
            for nm, h in (("ln_g", ln_g), ("ln_b", ln_b), ("ob", obb)):
                t = wp.tile(list(h.shape), h.dtype, tag=nm)
                nc.sync.dma_start(t[:], h[:])
                W[nm] = t
            ONEC = wp.tile([64, 64], f32, tag="onec")
            nc.vector.memset(ONEC[:], 1.0 / 64.0)

            # init persistent state: feature-major h arrives int8 with a
            # per-node f16 scale (halves the dominant host->device upload;
            # dequant here into RHS and NFM), [r|xyz] rows into NFM,
            # per-node 1/deg broadcast.
            with tc.tile_pool(name="hq", bufs=1) as hq:
                H8 = hq.tile([64, SHARD], i8, tag="H8")
                nc.sync.dma_start(H8[:], h8p[:])
                SCB = hq.tile([64, SHARD], f16, tag="SCB")
                nc.sync.dma_start(SCB[:], hsp[:].to_broadcast([64, SHARD]))
                nc.vector.tensor_copy(NFM[0:64, :], H8[:])
                nc.vector.tensor_tensor(NFM[0:64, :], NFM[0:64, :], SCB[:],
                                        Alu.mult)
                nc.vector.tensor_copy(RHS[0:64, :], NFM[0:64, :])
            # xyz rows only; row 64 (radial) is recomputed by every repack
            nc.sync.dma_start(NFM[65:68, :], rx0[:])
            nc.sync.dma_start(DINVB[64:68, :], dinvp[:].to_broadcast([4, SHARD]))
            # zero the per-channel trash rows of the dst-gather table once
            nc.sync.dma_start(
                Tsh[SHARD:TROWS, :], Z[:, 0:128])
            # fan the [16, IDXW] index uploads out to the SWDGE 128-row form
            for k in range(8):
                nc.sync.dma_start(sidxw[16 * k:16 * (k + 1), :], sidx16[:])
                nc.sync.dma_start(didxw[16 * k:16 * (k + 1), :], didx16[:])

            for l in range(L):
                if DBG >= 1 + 3 * l:
                    _repack_phase(nc, tc, bass, mybir, NFM, Tsh, Tfull, ACC, Z)
                if DBG >= 2 + 3 * l:
                    _edge_phase(nc, tc, bass, mybir, l, W, Tsh, Tfull, ACC,
                                sidxw, didxw)
                if DBG >= 3 + 3 * l:
                    _node_phase(nc, tc, bass, mybir, l, W, ACC, RHS, NFM, DINVB,
                                ONEC, outp)
                if DBG < 4 and l == 0:
                    break
            if DBG < 6:
                # ensure outputs are written so they exist
                with tc.tile_pool(name="dbgo", bufs=1) as dp:
                    zo = dp.tile([NOUT, 516], i8, tag="zo")
                    nc.vector.memset(zo[:], 0.0)
                    for k in range(SHARD // 512 + 1):
                        st = min(k * 512, SHARD + 4 - 516)
                        nc.sync.dma_start(outp[:, st:st + 516], zo[:])

    nc.finalize()
    return nc


def _edge_phase(nc, tc, bass, mybir, l, W, Tsh, Tfull, ACC, sidxw, didxw):
    from concourse import tile  # noqa: F401
    f16 = mybir.dt.float16
    f32 = mybir.dt.float32
    i16 = mybir.dt.int16
    AF = mybir.ActivationFunctionType
    Alu = mybir.AluOpType
    wa, wb, w1rs = W[f"wa{l}"], W[f"wb{l}"], W[f"w1rs{l}"]
    w2, cw1, cw2 = W[f"w2{l}"], W[f"cw1{l}"], W[f"cw2{l}"]
    b1, b2, cb1 = W[f"b1{l}"], W[f"b2{l}"], W[f"cb1{l}"]

    with (
        tc.tile_pool(name=f"eio{l}", bufs=2) as io,
        tc.tile_pool(name=f"emp{l}", bufs=2) as mp,
        tc.tile_pool(name=f"eps{l}", bufs=2, space=bass.MemorySpace.PSUM) as pp,
        tc.tile_pool(name=f"ep4{l}", bufs=2, space=bass.MemorySpace.PSUM) as p4p,
    ):
        ntl = min(NTL, DBG_TILES) if DBG_TILES else NTL
        for t in range(ntl):
            q = t // QT
            csl = slice(t * TPC, (t + 1) * TPC)
            sid = io.tile([128, TPC], i16, tag="sid")
            did = io.tile([128, TPC], i16, tag="did")
            nc.sync.dma_start(sid[:], sidxw[:, csl])
            nc.sync.dma_start(did[:], didxw[:, csl])

            Gs = io.tile([128, TILE], f16, tag="Gs")
            Gd = io.tile([128, TILE], f16, tag="Gd")
            for s in range(TILE // GSUB):
                isl = slice(s * (GSUB // 16), (s + 1) * (GSUB // 16))
                esl = slice(s * GSUB, (s + 1) * GSUB)
                nc.gpsimd.dma_gather(
                    Gs[:, esl].rearrange("p (a f) -> p a f", a=1),
                    Tfull[q * SUBT:(q + 1) * SUBT, :],
                    sid[:, isl], GSUB, GSUB, 128, transpose=True)
                nc.gpsimd.dma_gather(
                    Gd[:, esl].rearrange("p (a f) -> p a f", a=1),
                    Tsh[:], did[:, isl], GSUB, GSUB, 128, transpose=True)

            if EDGE_LVL < 2:
                continue
            # radial terms (feature-major) for MM1: row64 = rs+rd, 65:68 = xs*xd
            pxd = mp.tile([68, TILE], f16, tag="pxd")
            nc.vector.tensor_tensor(pxd[64:68, :], Gs[64:68, :], Gd[64:68, :], Alu.mult)
            nc.vector.tensor_tensor(pxd[64:65, :], Gs[64:65, :], Gd[64:65, :], Alu.add)

            # edge-major xyz via XBAR transpose
            XS = mp.tile([128, TILE // 128, 16], f16, tag="XS")
            XD = mp.tile([128, TILE // 128, 16], f16, tag="XD")
            nc.sync.dma_start(XS[:], Gs[64:80, :], transpose=True)
            nc.sync.dma_start(XD[:], Gd[64:80, :], transpose=True)
            dxT = mp.tile([128, TILE // 128, 3], f32, tag="dxT")
            nc.vector.tensor_tensor(dxT[:], XS[:, :, 1:4], XD[:, :, 1:4], Alu.subtract)
            sq = mp.tile([128, TILE // 128, 3], f32, tag="sq")
            nc.vector.tensor_tensor(sq[:], dxT[:], dxT[:], Alu.mult)
            rad = mp.tile([128, TILE // 128], f32, tag="rad")
            nc.vector.tensor_tensor(rad[:], sq[:, :, 0], sq[:, :, 1], Alu.add)
            nc.vector.tensor_tensor(rad[:], rad[:], sq[:, :, 2], Alu.add)
            sr = mp.tile([128, TILE // 128], f32, tag="sr")
            nc.scalar.activation(sr[:], rad[:], AF.Sqrt)
            nc.vector.tensor_scalar(sr[:], sr[:], 1e-30, None, Alu.add)
            inv = mp.tile([128, TILE // 128], f32, tag="inv")
            nc.vector.reciprocal(inv[:], sr[:])

            if EDGE_LVL < 3:
                continue
            MSG = mp.tile([64, TILE], f16, tag="MSG")
            CS16 = mp.tile([16, TILE], f16, tag="CS16")
            for k in range(NCHK):
                sl = slice(k * CH, (k + 1) * CH)
                P1 = pp.tile([64, CH], f32, tag="pb")
                nc.tensor.matmul(P1[:], wa[:], Gs[0:64, sl], start=True, stop=False)
                nc.tensor.matmul(P1[:], wb[:], Gd[0:64, sl], start=False, stop=False)
                nc.tensor.matmul(P1[:], w1rs[64:68, :], pxd[64:68, sl],
                                 start=False, stop=True, tile_position=(64, 0))
                S1 = mp.tile([64, CH], f16, tag="s1")
                nc.scalar.activation(S1[:], P1[:], AF.Silu, bias=b1[:])
                P2 = pp.tile([64, CH], f32, tag="pb")
                nc.tensor.matmul(P2[:], w2[:], S1[:], start=True, stop=True)
                nc.scalar.activation(MSG[:, sl], P2[:], AF.Silu, bias=b2[:])
                P3 = pp.tile([64, CH], f32, tag="pb")
                nc.tensor.matmul(P3[:], cw1[:], MSG[:, sl], start=True, stop=True)
                S3 = mp.tile([64, CH], f16, tag="s3")
                nc.scalar.activation(S3[:], P3[:], AF.Silu, bias=cb1[:])
                P4 = p4p.tile([1, CH], f32, tag="p4")
                nc.tensor.matmul(P4[:], cw2[:], S3[:], start=True, stop=True)
                nc.vector.tensor_copy(CS16[0:1, sl], P4[:])

            if EDGE_LVL < 4:
                continue
            CST = mp.tile([128, TILE // 128, 16], f16, tag="CST")
            nc.sync.dma_start(CST[:], CS16[:], transpose=True)
            cs32 = mp.tile([128, TILE // 128], f32, tag="cs32")
            nc.vector.tensor_copy(cs32[:], CST[:, :, 0])
            nc.vector.tensor_tensor(cs32[:], cs32[:], inv[:], Alu.mult)

            if EDGE_LVL < 5:
                continue
            PAY = mp.tile([128, TILE // 128, 128], f16, tag="PAY")
            nc.vector.tensor_tensor(
                PAY[:, :, 65:68], dxT[:],
                cs32[:].rearrange("p (a b) -> p a b", b=1).to_broadcast(
                    [128, TILE // 128, 3]),
                Alu.mult)
            nc.sync.dma_start(PAY[:, :, 0:64], MSG[:], transpose=True)
            for s in range(TILE // GSUB):
                nc.gpsimd.dma_scatter_add(
                    ACC[:], PAY[:, s * (GSUB // 128):(s + 1) * (GSUB // 128), :],
                    did[:, s * (GSUB // 16):(s + 1) * (GSUB // 16)],
                    GSUB, GSUB, 128)


def _node_phase(nc, tc, bass, mybir, l, W, ACC, RHS, NFM, DINVB, ONEC, outp):
    f16 = mybir.dt.float16
    f32 = mybir.dt.float32
    i8 = mybir.dt.int8
    AF = mybir.ActivationFunctionType
    Alu = mybir.AluOpType
    nw1, nw2 = W[f"nw1{l}"], W[f"nw2{l}"]
    nb1, nb2 = W[f"nb1{l}"], W[f"nb2{l}"]

    with (
        tc.tile_pool(name=f"nd{l}", bufs=3) as nd,
        tc.tile_pool(name=f"no{l}", bufs=1) as op_,
        tc.tile_pool(name=f"nps{l}", bufs=2, space=bass.MemorySpace.PSUM) as pp,
        tc.tile_pool(name=f"np2{l}", bufs=2, space=bass.MemorySpace.PSUM) as pp2,
    ):
        OALL = None
        if l == L - 1:
            OALL = op_.tile([NOUT, SHARD], f16, tag="OALL")
        for b in range(NB):
            bsl = slice(b * 128, (b + 1) * 128)
            if USE_XBAR_DRAM:
                ABT = nd.tile([128, 128], f16, tag="ABT")
                nc.sync.dma_start(ABT[:], ACC[bsl, :], transpose=True)
            else:
                AB = nd.tile([128, 128], f16, tag="AB")
                nc.sync.dma_start(AB[:], ACC[bsl, :])
                ABT = nd.tile([128, 128], f16, tag="ABT")
                nc.sync.dma_start(ABT[:], AB[:], transpose=True)
            nc.sync.dma_start(RHS[64:128, bsl], ABT[0:64, :])
            if l == 0:
                xn = nd.tile([68, 128], f16, tag="xn")
                nc.vector.tensor_tensor(
                    xn[64:68, :], ABT[64:68, :], DINVB[64:68, bsl], Alu.mult)
                nc.vector.tensor_tensor(
                    NFM[64:68, bsl], NFM[64:68, bsl], xn[64:68, :], Alu.add)

        # node MLP + gelu + LN (+ output head on last layer)
        nchunks = [(k * MLPCH, MLPCH) for k in range(NMLP)]
        if NMLP * MLPCH < SHARD:
            nchunks.append((NMLP * MLPCH, SHARD - NMLP * MLPCH))
        for (st, ln_) in nchunks:
            sl = slice(st, st + ln_)
            P = pp.tile([64, ln_], f32, tag="pn")
            nc.tensor.matmul(P[:], nw1[:], RHS[:, sl], start=True, stop=True)
            S = nd.tile([64, ln_], f16, tag="sn")
            nc.scalar.activation(S[:], P[:], AF.Silu, bias=nb1[:])
            P2 = pp.tile([64, ln_], f32, tag="pn")
            nc.tensor.matmul(P2[:], nw2[:], S[:], start=True, stop=True)
            H2 = nd.tile([64, ln_], f32, tag="h2")
            nc.scalar.activation(H2[:], P2[:], AF.Gelu, bias=nb2[:])
            SQH = nd.tile([64, ln_], f32, tag="sqh")
            nc.scalar.activation(SQH[:], H2[:], AF.Square)
            MUR = pp2.tile([64, ln_], f32, tag="mur")
            nc.tensor.matmul(MUR[:], ONEC[:], H2[:], start=True, stop=True)
            MU = nd.tile([64, ln_], f32, tag="mu")
            nc.vector.tensor_copy(MU[:], MUR[:])
            E2R = pp2.tile([64, ln_], f32, tag="mur")
            nc.tensor.matmul(E2R[:], ONEC[:], SQH[:], start=True, stop=True)
            VAR = nd.tile([64, ln_], f32, tag="var")
            nc.vector.tensor_tensor(VAR[:], MU[:], MU[:], Alu.mult)
            nc.vector.tensor_tensor(VAR[:], E2R[:], VAR[:], Alu.subtract)
            nc.vector.tensor_scalar(VAR[:], VAR[:], 1e-5, None, Alu.add)
            SD = nd.tile([64, ln_], f32, tag="sd")
            nc.scalar.activation(SD[:], VAR[:], AF.Sqrt)
            ISD = nd.tile([64, ln_], f32, tag="isd")
            nc.vector.reciprocal(ISD[:], SD[:])
            HC = nd.tile([64, ln_], f32, tag="hc")
            nc.vector.tensor_tensor(HC[:], H2[:], MU[:], Alu.subtract)
            nc.vector.tensor_tensor(HC[:], HC[:], ISD[:], Alu.mult)
            HFIN = nd.tile([64, ln_], f32, tag="hfin")
            nc.vector.tensor_scalar(
                HFIN[:], HC[:], W["ln_g"][:], W["ln_b"][:], Alu.mult, Alu.add)
            if l == 0:
                nc.vector.tensor_copy(NFM[0:64, sl], HFIN[:])
                nc.vector.tensor_copy(RHS[0:64, sl], HFIN[:])
            else:
                HF16 = nd.tile([64, ln_], f16, tag="hf16")
                nc.vector.tensor_copy(HF16[:], HFIN[:])
                PO = pp.tile([NOUT, ln_], f32, tag="po")
                nc.tensor.matmul(PO[:], W["ow"][:], HF16[:], start=True, stop=True)
                nc.scalar.activation(OALL[:, sl], PO[:], AF.Identity,
                                     bias=W["ob"][:])

        if l == L - 1:
            # per-channel abs-max -> symmetric int8 (round-to-nearest,
            # saturating); scale rides in outp cols SHARD:SHARD+4 (f32 bitcast)
            MX = op_.tile([NOUT, 1], f32, tag="MX")
            nc.vector.tensor_reduce(MX[:], OALL[:], mybir.AxisListType.X,
                                    Alu.max, apply_absolute_value=True)
            nc.vector.tensor_scalar(MX[:], MX[:], 1e-20, None, Alu.max)
            SC = op_.tile([NOUT, 1], f32, tag="SC")
            nc.vector.reciprocal(SC[:], MX[:])
            nc.vector.tensor_scalar(SC[:], SC[:], 127.0, None, Alu.mult)
            OI8 = op_.tile([NOUT, SHARD], i8, tag="OI8")
            nc.vector.tensor_scalar(OI8[:], OALL[:], SC[:], None, Alu.mult)
            nc.sync.dma_start(outp[:, 0:SHARD], OI8[:])
            nc.sync.dma_start(outp[:, SHARD:SHARD + 4], MX[:].bitcast(i8))


def _repack_phase(nc, tc, bass, mybir, NFM, Tsh, Tfull, ACC, Z):
    f16 = mybir.dt.float16
    f32 = mybir.dt.float32
    Alu = mybir.AluOpType
    with tc.tile_pool(name="pk", bufs=1) as pk:
        PK = pk.tile([128, NB, 128], f16, tag="PK")
        nc.vector.memset(PK[:], 0.0)
        nc.sync.dma_start(PK[:, :, 0:80], NFM[:, :], transpose=True)
        TSQ = pk.tile([128, NB, 3], f32, tag="TSQ")
        nc.vector.tensor_tensor(TSQ[:], PK[:, :, 65:68], PK[:, :, 65:68], Alu.mult)
        R2 = pk.tile([128, NB], f32, tag="R2")
        nc.vector.tensor_tensor(R2[:], TSQ[:, :, 0], TSQ[:, :, 1], Alu.add)
        nc.vector.tensor_tensor(R2[:], R2[:], TSQ[:, :, 2], Alu.add)
        nc.vector.tensor_copy(PK[:, :, 64], R2[:])
        nc.sync.dma_start(
            Tsh[0:SHARD, :].rearrange("(b p) e -> p b e", p=128), PK[:])
        for i in range(8):
            nc.sync.dma_start(
                ACC[:].rearrange("(a r) e -> a (r e)", a=8)[i:i + 1, :]
                .rearrange("a (p f) -> (a p) f", p=128),
                Z[:])
        nc.gpsimd.collective_compute(
            "AllGather", Alu.bypass,
            replica_groups=[list(range(NCORES))],
            ins=[Tsh[0:SHARD, :].opt()],
            outs=[Tfull[:].opt()],
        )


# ---------------- host runner (cached PJRT executable) ----------------
_RUNNER = None
_NC = None


def _get_nc():
    global _NC
    if _NC is None:
        _NC = _build_bass()
    return _NC


def _get_runner():
    """Build the Bass module once, lower it through the bass_exec custom
    call, and cache the jitted SPMD executable. run_bass_kernel_spmd's
    axon path (bass2jax.run_bass_via_pjrt) constructs a fresh jax.jit on
    every call, re-paying trace + XLA compile + NEFF ship per launch;
    this is the same lowering with the jit hoisted. No donated zero
    output buffers: outp is fully written by the kernel."""
    global _RUNNER
    if _RUNNER is not None:
        return _RUNNER

    import jax
    from jax.experimental.shard_map import shard_map
    from jax.sharding import Mesh, PartitionSpec

    import concourse.bass2jax as b2j
    import concourse.mybir as mybir

    nc = _get_nc()
    b2j.install_neuronx_cc_hook()

    partition_name = nc.partition_id_tensor.name if nc.partition_id_tensor else None
    in_names, out_names, out_avals = [], [], []
    for alloc in nc.m.functions[0].allocations:
        if not isinstance(alloc, mybir.MemoryLocationSet):
            continue
        name = alloc.memorylocations[0].name
        if alloc.kind == "ExternalInput":
            if name != partition_name:
                in_names.append(name)
        elif alloc.kind == "ExternalOutput":
            out_avals.append(jax.core.ShapedArray(
                tuple(alloc.tensor_shape), mybir.dt.np(alloc.dtype)))
            out_names.append(name)
    bind_names = tuple(in_names) + ((partition_name,) if partition_name else ())

    def _body(*args):
        operands = list(args)
        if partition_name is not None:
            operands.append(b2j.partition_id_tensor())
        return tuple(b2j._bass_exec_p.bind(
            *operands,
            out_avals=tuple(out_avals),
            in_names=bind_names,
            out_names=tuple(out_names),
            lowering_input_output_aliases=(),
            sim_require_finite=True,
            sim_require_nnan=True,
            nc=nc,
        ))

    devices = jax.devices()[:NCORES]
    assert len(devices) == NCORES, f"need {NCORES} cores, have {len(jax.devices())}"
    mesh = Mesh(np.asarray(devices), ("core",))
    sharded = jax.jit(
        shard_map(_body, mesh=mesh,
                  in_specs=(PartitionSpec("core"),) * len(in_names),
                  out_specs=(PartitionSpec("core"),) * len(out_names),
                  check_rep=False),
        keep_unused=True)
    _RUNNER = (sharded, in_names, out_names)
    return _RUNNER


def _run_device(in_maps):
    if isinstance(in_maps, dict):
        per_core, concat = in_maps["maps"], in_maps["concat"]
    else:
        per_core, concat = in_maps, None
    try:
        sharded, in_names, out_names = _get_runner()
        if concat is None:
            concat = {
                name: np.concatenate(
                    [np.asarray(per_core[c][name]) for c in range(NCORES)],
                    axis=0)
                for name in in_names
            }
        out_arrs = sharded(*[concat[name] for name in in_names])
        raw = np.asarray(out_arrs[out_names.index("outp")])  # [8*32, SHARD+4] i8
    except Exception:
        # cached-jit path failed (internal bass2jax API drift?) — fall back
        # to the stock per-call runner; slower but identical device program
        from concourse.bass_utils import run_bass_kernel_spmd
        res = run_bass_kernel_spmd(_get_nc(), per_core,
                                   core_ids=list(range(NCORES)))
        raw = np.concatenate(
            [np.asarray(res.results[c]["outp"]) for c in range(NCORES)], axis=0)
    scl = np.ascontiguousarray(raw[:, SHARD:SHARD + 4]).view(np.float32)
    deq = raw[:, :SHARD].astype(np.float32)
    deq *= scl / 127.0
    out = deq.reshape(NCORES, NOUT, SHARD).transpose(0, 2, 1).reshape(NSH, NOUT)
    return out[:N]


def _pack_inputs(node_feat, xyz, src, dst, weights):
    """Host-side preprocessing -> per-core in_maps."""
    (edge_w1, edge_b1, edge_w2, edge_b2, coord_w1, coord_b1, coord_w2,
     node_w1, node_b1, node_w2, node_b2, ln_g, ln_b, out_w, out_b) = weights

    # per-node symmetric int8 quantization of h (dequantized on device)
    hsc = np.maximum(np.abs(node_feat).max(1, keepdims=True) / 127.0, 1e-8)
    hq8 = np.clip(np.round(node_feat / hsc), -127, 127).astype(np.int8)
    x16 = xyz.astype(np.float16)
    deg = np.bincount(dst, minlength=N).astype(np.float32)
    dinv = (1.0 / np.maximum(deg, 1.0)).astype(np.float16)

    # pack the f16 weight blob once; each core ships 1/8th of it
    wblob = np.zeros(WTOT, np.float16)
    wsrc = {}
    for l in range(L):
        w1 = edge_w1[l].astype(np.float32)
        wr = w1[128]
        wsrc[f"wa{l}"] = w1[0:64]
        wsrc[f"wb{l}"] = w1[64:128]
        wsrc[f"w1rs{l}"] = np.concatenate(
            [wr[None], np.tile((-2.0 * wr)[None], (3, 1))], 0)
        wsrc[f"w2{l}"] = edge_w2[l]
        wsrc[f"cw1{l}"] = coord_w1[l]
        wsrc[f"cw2{l}"] = coord_w2[l]
        wsrc[f"nw1{l}"] = node_w1[l]
        wsrc[f"nw2{l}"] = node_w2[l]
    wsrc["ow"] = out_w
    for name, (off, shp) in _WOFF.items():
        wblob[off:off + shp[0] * shp[1]] = (
            wsrc[name].astype(np.float16).reshape(-1))

    core = dst // SHARD
    qq = src // SUBT
    chan = dst % 16
    order = np.lexsort((dst, chan, qq, core))
    src_s, dst_s = src[order], dst[order]

    # group boundaries per (core, q, chan)
    key = (core[order] * 4 + qq[order]) * 16 + chan[order]
    cnt = np.bincount(key, minlength=NCORES * 4 * 16)
    assert cnt.max() <= QT * TPC, f"group overflow: {cnt.max()} > {QT * TPC}"
    starts = np.concatenate(([0], np.cumsum(cnt)))

    in_maps = []
    for c in range(NCORES):
        lo = c * SHARD
        hi = min(lo + SHARD, N)
        nreal = hi - lo

        rx0 = np.zeros((3, SHARD), np.float16)
        rx0[:, :nreal] = x16[lo:hi].T

        h8 = np.zeros((64, SHARD), np.int8)
        h8[:, :nreal] = hq8[lo:hi].T
        hs = np.zeros((1, SHARD), np.float16)
        hs[0, :nreal] = hsc[lo:hi, 0].astype(np.float16)

        dv = np.ones((1, SHARD), np.float16)
        dv[0, :nreal] = dinv[lo:hi]

        # idx packing: A[tile, chan(16), pos(TPC)]
        As = np.zeros((NTL, 16, TPC), np.int16)
        Ad = np.full((NTL, 16, TPC), 0, np.int16)
        for q in range(4):
            for ch in range(16):
                g = (c * 4 + q) * 16 + ch
                e0, e1 = starts[g], starts[g] + cnt[g]
                sl_src = (src_s[e0:e1] - q * SUBT).astype(np.int16)
                sl_dst = (dst_s[e0:e1] - lo).astype(np.int16)
                nfull = cnt[g]
                base_t = q * QT
                # Round-robin each dst's edges over all QT*8 (tile,
                # scatter-subcall) slots. Edges of one dst are consecutive
                # in the sorted order, so occurrences land in distinct
                # tiles (k<=13) and beyond that in distinct subcalls >=32
                # descriptors apart in the channel FIFO — duplicate idx
                # within one dma_scatter_add call race the f16 RMW
                # pipeline (measured: ~150 corrupted nodes/run without
                # this).
                NSLOT = QT * NCHK          # 104
                BCAP = TPC // NCHK         # 32 slots per (tile, subcall)
                As[base_t:base_t + QT, ch, :] = 0
                Ad[base_t:base_t + QT, ch, :] = SHARD + ch
                if nfull > 0:
                    s_ = np.arange(nfull) % NSLOT
                    t_of, u_of = s_ % QT, s_ // QT
                    # rank within each (tile, subcall) bucket
                    bucket = t_of * NCHK + u_of
                    order2 = np.argsort(bucket, kind="stable")
                    bsort = bucket[order2]
                    bstart = np.searchsorted(bsort, np.arange(NSLOT))
                    rank = np.arange(nfull) - bstart[bsort]
                    assert rank.max() < BCAP, rank.max()
                    col = u_of[order2] * BCAP + rank
                    As[base_t + t_of[order2], ch, col] = sl_src[order2]
                    Ad[base_t + t_of[order2], ch, col] = sl_dst[order2]
        # [16, IDXW]: rows = 16-wrap; tile t occupies cols [t*TPC, (t+1)*TPC)
        Ws_ = As.transpose(1, 0, 2).reshape(16, NTL * TPC)
        Wd_ = Ad.transpose(1, 0, 2).reshape(16, NTL * TPC)

        m = {
            "rx0": rx0, "h8": h8, "hs": hs, "dinv": dv,
            "sidx16": np.ascontiguousarray(Ws_),
            "didx16": np.ascontiguousarray(Wd_),
            "wsh": wblob[c * WSH_LEN:(c + 1) * WSH_LEN].reshape(1, WSH_LEN),
            "ln_g": ln_g.reshape(64, 1).astype(np.float32),
            "ln_b": ln_b.reshape(64, 1).astype(np.float32),
            "ob": out_b.reshape(NOUT, 1).astype(np.float32),
        }
        for l in range(L):
            m[f"b1{l}"] = edge_b1[l].reshape(64, 1).astype(np.float32)
            m[f"b2{l}"] = edge_b2[l].reshape(64, 1).astype(np.float32)
            m[f"cb1{l}"] = coord_b1[l].reshape(64, 1).astype(np.float32)
            m[f"nb1{l}"] = node_b1[l].reshape(64, 1).astype(np.float32)
            m[f"nb2{l}"] = node_b2[l].reshape(64, 1).astype(np.float32)
        in_maps.append(m)
    # pre-concatenate into the shard_map global layout (untimed host prep)
    concat = {
        name: np.ascontiguousarray(
            np.concatenate([in_maps[c][name] for c in range(NCORES)], axis=0))
        for name in in_maps[0]
    }
    return {"maps": in_maps, "concat": concat}


def kernel(node_feat, xyz, src, dst, edge_w1, edge_b1, edge_w2, edge_b2,
           coord_w1, coord_b1, coord_w2, node_w1, node_b1, node_w2, node_b2,
           ln_g, ln_b, out_w, out_b):
    node_feat = np.asarray(node_feat, np.float32)
    xyz = np.asarray(xyz, np.float32)
    src = np.asarray(src, np.int32)
    dst = np.asarray(dst, np.int32)
    weights = (np.asarray(edge_w1, np.float32), np.asarray(edge_b1, np.float32),
               np.asarray(edge_w2, np.float32), np.asarray(edge_b2, np.float32),
               np.asarray(coord_w1, np.float32), np.asarray(coord_b1, np.float32),
               np.asarray(coord_w2, np.float32),
               np.asarray(node_w1, np.float32), np.asarray(node_b1, np.float32),
               np.asarray(node_w2, np.float32), np.asarray(node_b2, np.float32),
               np.asarray(ln_g, np.float32), np.asarray(ln_b, np.float32),
               np.asarray(out_w, np.float32), np.asarray(out_b, np.float32))
    in_maps = _pack_inputs(node_feat, xyz, src, dst, weights)
    return _run_device(in_maps)
